# revision 16
# baseline (speedup 1.0000x reference)
"""BertSelfAttention Trainium2 kernel (8-core SPMD, head-parallel).

Problem: B=2, S=2048, D=768, H=12, Dh=64.
Outputs: (context [B,S,768] f32, attention_softmax [B,H,S,S] f32).

Sharding: 24 (b,h) pairs -> 3 heads per core. Cores 0-3: batch 0,
cores 4-7: batch 1.

Per-core dataflow:
  qT[dh,S] / kT[dh,S] = W^T @ h^T  (contraction over D in chunks of 128; an
      optional extra chunk holds a ones-row in h^T against a bias-row in W,
      so biases are exact). Heads are packed in PAIRS across SBUF partition
      row-groups: (h0,h1) and (h2,h2-duplicate), enabling PE row-tiling so
      the two K=64 scores matmuls of a pair run CONCURRENTLY in array rows
      0-63 / 64-127.
  v[s,dh] with an appended ones column (v_aug[:,64]=1).
  scoresT[k,q] = kT^T @ qT  (PE; transposed layout)
  e = exp(scoresT + mask[k])  (ACT; mask as per-partition bias; the
      1/sqrt(dh) scale is folded into Wq on the host)
  ctx_unT[dh+1, q] = v_aug^T @ e  (PE; row 64 = softmax denominators)
  recip = exp(-ln(sums))  (ACT, shared table set with exp)
  softmaxT = e * bcast(recip)  (rank-1 PE broadcast + DVE multiply)
Outputs are written transposed ([h,k,q] / [h,dh+1,q]); the host undoes the
transpose with a view and normalizes the context rows by the sums row.

The schedule runs 6 uniform sub-phases, each covering (headA, q-quarterA)
in PE row-group 0 and (headB, q-quarterB) in row-group 1, with the previous
sub-phase's normalize+DMA and the next heads' projections interleaved.
"""

import numpy as np
import ml_dtypes

import concourse.bass as bass
import concourse.tile as tile
from concourse import bacc, mybir
from concourse.bass_utils import run_bass_kernel_spmd

P = 128
S = 2048
D = 768
DH = 64
NH = 3  # heads per core
KT = S // P  # 16 k-tiles
NQ = S // 512  # 4 q-quarters
OC = 7  # max contraction chunks: 6 data + 1 bias
AF = mybir.ActivationFunctionType
BF16 = mybir.dt.bfloat16
F32 = mybir.dt.float32

_CACHE = {}


def _build_program(scratch_out=False, reps=1, loop_reps=1, has_bias=True):
    """scratch_out=True: outputs go to Internal DRAM (not downloaded) and a
    tiny dummy output is returned instead — used for wall-clock timing with
    the body repeated `reps` times (static unroll) or `loop_reps` times
    (hardware For_i loop) inside one dispatch."""
    nc = bacc.Bacc(None, target_bir_lowering=False)
    ht = nc.dram_tensor("ht", [OC * P, S], BF16, kind="ExternalInput")
    wq = nc.dram_tensor("wq", [OC * P, NH * DH], BF16, kind="ExternalInput")
    wk = nc.dram_tensor("wk", [OC * P, NH * DH], BF16, kind="ExternalInput")
    wv = nc.dram_tensor("wv", [OC * P, NH * DH], BF16, kind="ExternalInput")
    maskc = nc.dram_tensor("maskc", [P, KT], F32, kind="ExternalInput")
    if scratch_out:
        sm = nc.dram_tensor("sm", [NH, S, S], BF16)
        ctx = nc.dram_tensor("ctx", [NH, DH + 1, S], F32)
        dummy = nc.dram_tensor("tdummy", [1, 8], F32, kind="ExternalOutput")
    else:
        sm = nc.dram_tensor("sm", [NH, S, S], BF16, kind="ExternalOutput")
        ctx = nc.dram_tensor("ctx", [NH, DH + 1, S], F32, kind="ExternalOutput")
        dummy = None

    with tile.TileContext(nc) as tc:
        with (
            tc.tile_pool(name="const", bufs=1) as const,
            tc.tile_pool(name="exp", bufs=34) as exppool,
            tc.tile_pool(name="smt", bufs=4) as smpool,
            tc.tile_pool(name="bc", bufs=3) as bcpool,
            tc.tile_pool(name="ctxs", bufs=3) as ctxpool,
            tc.tile_pool(name="small", bufs=3) as smallsb,
            tc.tile_pool(name="ps_sc", bufs=2, space="PSUM") as ps_sc,
            tc.tile_pool(name="ps_proj", bufs=2, space="PSUM") as ps_proj,
            tc.tile_pool(name="ps_ctx", bufs=2, space="PSUM") as ps_ctx,
        ):
            # ---- persistent tiles ----
            ht_sb = const.tile([P, OC, S], BF16, tag="ht")
            w_sb = {}
            for name in ("wq", "wk", "wv"):
                w_sb[name] = const.tile(
                    [P, OC, NH * DH], BF16, tag=name, name=name
                )
            maskc_sb = const.tile([P, KT], F32, tag="maskc")
            ones_sb = const.tile([1, P], BF16, tag="ones")
            nc.vector.memset(ones_sb[:], 1.0)
            # q/k per virtual pair: vp0 = (h0 rows 0-63 | h1 rows 64-127),
            # vp1 = (h2 | h2 duplicate). [128, vp, {q,k}, S]
            qk = const.tile([P, 2, 2, S], BF16, tag="qk")
            # v with ones column: [128(k within tile), kt, head, 65]
            v_all = const.tile([P, KT, NH, DH + 1], BF16, tag="vall")
            nc.vector.memset(v_all[:, :, :, DH], 1.0)
            if scratch_out:
                dummy_sb = const.tile([1, 8], F32, tag="dummy")
                nc.vector.memset(dummy_sb[:], 0.0)

            def body():
                _attention_body(
                    nc, tc, ht, wq, wk, wv, maskc, sm, ctx,
                    ht_sb, w_sb, maskc_sb, ones_sb, qk, v_all,
                    exppool, smpool, bcpool, ctxpool, smallsb,
                    ps_sc, ps_proj, ps_ctx, has_bias,
                )

            if loop_reps > 1:
                with tc.For_i(0, loop_reps, 1):
                    body()
            else:
                for _rep in range(reps):
                    body()
            if scratch_out:
                nc.sync.dma_start(dummy[:], dummy_sb[:])

    nc.compile()
    return nc


def _attention_body(
    nc, tc, ht, wq, wk, wv, maskc, sm, ctx,
    ht_sb, w_sb, maskc_sb, ones_sb, qk, v_all,
    exppool, smpool, bcpool, ctxpool, smallsb, ps_sc, ps_proj, ps_ctx,
    has_bias,
):
    OCU = OC if has_bias else OC - 1  # contraction chunks actually used

    # ---- loads (split per contraction chunk so compute starts early) ----
    htr = ht.rearrange("(o p) f -> p o f", p=P)
    for o in range(OCU):
        nc.sync.dma_start(ht_sb[:, o], htr[:, o])
    for name, t in (("wq", wq), ("wk", wk), ("wv", wv)):
        nc.sync.dma_start(w_sb[name][:], t.rearrange("(o p) f -> p o f", p=P))
    nc.sync.dma_start(maskc_sb[:], maskc[:])

    # virtual pairs: row-group 0 / row-group 1 head of each
    VP_HEADS = ((0, 1), (2, 2))

    def proj_qk_unit(vp, which, wname, qc):
        """[128, 512] chunk: col-tiled pair projection into both row-groups."""
        hA, hB = VP_HEADS[vp]
        qsl = slice(qc * 512, (qc + 1) * 512)
        pt = ps_proj.tile([P, 512], F32, tag="psp")
        for o in range(OCU):
            nc.tensor.matmul(
                pt[0:DH],
                lhsT=w_sb[wname][:, o, hA * DH : (hA + 1) * DH],
                rhs=ht_sb[:, o, qsl],
                start=(o == 0),
                stop=(o == OCU - 1),
            )
        for o in range(OCU):
            nc.tensor.matmul(
                pt[DH:P],
                lhsT=w_sb[wname][:, o, hB * DH : (hB + 1) * DH],
                rhs=ht_sb[:, o, qsl],
                start=(o == 0),
                stop=(o == OCU - 1),
            )
        nc.vector.tensor_copy(qk[:, vp, which, qsl], pt[:])

    def proj_v_unit(kt):
        pv = ps_proj.tile([P, 512], F32, tag="psp")
        for o in range(OCU):
            nc.tensor.matmul(
                pv[:, : NH * DH],
                lhsT=ht_sb[:, o, kt * P : (kt + 1) * P],
                rhs=w_sb["wv"][:, o, :],
                start=(o == 0),
                stop=(o == OCU - 1),
            )
        nc.vector.tensor_copy(
            v_all[:, kt, :, 0:DH],
            pv[:, : NH * DH].rearrange("p (a b) -> p a b", a=NH),
        )

    def kloop_unit(ph, kt, pctxA, pctxB, et):
        """Row-tiled pair: scores+exp+ctx for (headA, qA) and (headB, qB)."""
        vp, hA, qA, hB, qB = ph
        ksl = slice(kt * P, (kt + 1) * P)
        pss = ps_sc.tile([P, 1024], F32, tag="ps")
        nc.tensor.matmul(
            pss[:, 0:512],
            lhsT=qk[0:DH, vp, 1, ksl],
            rhs=qk[0:DH, vp, 0, qA * 512 : (qA + 1) * 512],
            start=True, stop=True,
        )
        nc.tensor.matmul(
            pss[:, 512:1024],
            lhsT=qk[DH:P, vp, 1, ksl],
            rhs=qk[DH:P, vp, 0, qB * 512 : (qB + 1) * 512],
            start=True, stop=True,
        )
        nc.scalar.activation(
            et[:], pss[:], AF.Exp, bias=maskc_sb[:, kt : kt + 1]
        )
        nc.tensor.matmul(
            pctxA[:], lhsT=v_all[:, kt, hA], rhs=et[:, 0:512],
            start=(kt == 0), stop=(kt == KT - 1),
        )
        nc.tensor.matmul(
            pctxB[:], lhsT=v_all[:, kt, hB], rhs=et[:, 512:1024],
            start=(kt == 0), stop=(kt == KT - 1),
        )

    def side_tail(h, q, pctx, bcast, side):
        """recip of denominators, ctx out, broadcast into bcast half."""
        qsl = slice(q * 512, (q + 1) * 512)
        lnrow = smallsb.tile([1, 512], F32, tag="lnrow")
        nc.scalar.activation(lnrow[:], pctx[DH : DH + 1, :], AF.Ln)
        recip = smallsb.tile([1, 512], BF16, tag="recip")
        nc.scalar.activation(recip[:], lnrow[:], AF.Exp, scale=-1.0)

        ctxs = ctxpool.tile([DH + 1, 512], F32, tag="ctxs")
        nc.vector.tensor_copy(ctxs[:], pctx[:])
        nc.sync.dma_start(ctx[h, :, qsl], ctxs[:])

        pb = ps_proj.tile([P, 512], F32, tag="psp")
        nc.tensor.matmul(
            pb[:], lhsT=ones_sb[:], rhs=recip[:], start=True, stop=True
        )
        nc.vector.tensor_copy(bcast[:, side * 512 : (side + 1) * 512], pb[:])

    def normalize_unit(ph, kt, et, bcast):
        vp, hA, qA, hB, qB = ph
        smt = smpool.tile([P, 1024], BF16, tag="smt")
        nc.vector.tensor_mul(smt[:], et[:], bcast[:])
        nc.sync.dma_start(
            sm[hA, kt * P : (kt + 1) * P, qA * 512 : (qA + 1) * 512],
            smt[:, 0:512],
        )
        nc.sync.dma_start(
            sm[hB, kt * P : (kt + 1) * P, qB * 512 : (qB + 1) * 512],
            smt[:, 512:1024],
        )

    # phases: (vp, headA, qA, headB, qB)
    phases = [(0, 0, qq, 1, qq) for qq in range(NQ)]
    phases += [(1, 2, 2 * i, 2, 2 * i + 1) for i in range(2)]

    # vp0 projections up front
    for qc in range(NQ):
        for which, wname in ((0, "wq"), (1, "wk")):
            proj_qk_unit(0, which, wname, qc)

    prev = None  # (phase, exp_tiles, bcast)
    for pi, ph in enumerate(phases):
        vp, hA, qA, hB, qB = ph
        pctxA = ps_ctx.tile([DH + 1, 512], F32, tag="pctx")
        pctxB = ps_ctx.tile([DH + 1, 512], F32, tag="pctx")
        exp_tiles = []
        extra = []
        if pi in (2, 3):  # vp1 projections spread over phases 2-3
            extra += [
                (lambda w=w0, n=wn, q=q: proj_qk_unit(1, w, n, q))
                for q in (range(0, 2) if pi == 2 else range(2, 4))
                for w0, wn in ((0, "wq"), (1, "wk"))
            ]
        if prev is not None:
            pph, pe, pb = prev
            extra += [
                (lambda k=kt, a=pph, e=pe[kt], b=pb: normalize_unit(a, k, e, b))
                for kt in range(KT)
            ]
        emitted = 0
        for kt in range(KT):
            if pi == 0:
                proj_v_unit(kt)  # just-in-time for the ctx matmuls of phase 0
            et = exppool.tile([P, 1024], BF16, tag="exp")
            kloop_unit(ph, kt, pctxA, pctxB, et)
            exp_tiles.append(et)
            due = (kt + 1) * len(extra) // KT
            while emitted < due:
                extra[emitted]()
                emitted += 1
        while emitted < len(extra):
            extra[emitted]()
            emitted += 1
        bcast = bcpool.tile([P, 1024], BF16, tag="bc")
        side_tail(hA, qA, pctxA, bcast, 0)
        side_tail(hB, qB, pctxB, bcast, 1)
        prev = (ph, exp_tiles, bcast)

    pph, pe, pb = prev
    for kt in range(KT):
        normalize_unit(pph, kt, pe[kt], pb)


def _prep_inputs(hidden_states, attention_mask, Wq, bq, Wk, bk, Wv, bv):
    """Host-side shard prep. Returns (in_maps list of 8 dicts, has_bias)."""
    B = hidden_states.shape[0]
    bf = ml_dtypes.bfloat16
    has_bias = bool(
        np.any(np.asarray(bq)) or np.any(np.asarray(bk)) or np.any(np.asarray(bv))
    )
    hta = {}
    maskc = {}
    for b in range(B):
        m = np.zeros((OC * P, S), np.float32)
        m[:D] = np.asarray(hidden_states[b]).T
        m[D] = 1.0
        hta[b] = m.astype(bf)
        maskc[b] = np.ascontiguousarray(
            np.asarray(attention_mask[b, 0, 0], np.float32).reshape(KT, P).T
        )

    def wslice(W, bias, cols, scale=1.0):
        m = np.zeros((OC * P, NH * DH), np.float32)
        m[:D] = np.asarray(W[:, cols], np.float32) * scale
        m[D] = np.asarray(bias[cols], np.float32) * scale
        return m.astype(bf)

    in_maps = []
    for c in range(8):
        b = c // 4
        h0 = (c % 4) * NH
        cols = slice(h0 * DH, (h0 + NH) * DH)
        in_maps.append(
            {
                "ht": hta[b],
                "wq": wslice(Wq, bq, cols, scale=1.0 / np.sqrt(DH)),
                "wk": wslice(Wk, bk, cols),
                "wv": wslice(Wv, bv, cols),
                "maskc": maskc[b],
            }
        )
    return in_maps, has_bias


def kernel(hidden_states, attention_mask, Wq, bq, Wk, bk, Wv, bv):
    in_maps, has_bias = _prep_inputs(
        hidden_states, attention_mask, Wq, bq, Wk, bk, Wv, bv
    )
    key = ("real", has_bias)
    if key not in _CACHE:
        _CACHE[key] = _build_program(has_bias=has_bias)
    nc = _CACHE[key]

    res = run_bass_kernel_spmd(nc, in_maps, core_ids=list(range(8)))

    B, H = 2, 12
    softmax = np.empty((B, H, S, S), np.float32)
    context = np.empty((B, S, H * DH), np.float32)
    for c in range(8):
        b = c // 4
        h0 = (c % 4) * NH
        smc = np.asarray(res.results[c]["sm"])  # [NH, S(k), S(q)] bf16
        ctc = np.asarray(res.results[c]["ctx"])  # [NH, DH+1, S(q)] f32
        for j in range(NH):
            softmax[b, h0 + j] = smc[j].T
            context[b, :, (h0 + j) * DH : (h0 + j + 1) * DH] = (
                ctc[j, :DH] / ctc[j, DH : DH + 1]
            ).T
    return context, softmax
